# revision 2
# baseline (speedup 1.0000x reference)
"""EdgeConv GNN message passing on 8 TRN2 cores — v2.

Math per edge e (endpoints row[e], col[e]):
    out0 = edge_attr @ w_self
    out  = out0 * (1 + 0.5*x[row]@w_h + 0.5*x[col]@w_t) + edge_attr
    out  = relu(batchnorm(out))        # BN stats over ALL edges

Key ideas vs v1:
  * x[row] @ w_h == (x @ w_h)[row]: precompute node tables H = 0.5*x@w_h,
    T = 0.5*x@w_t on device (xbar-transposed bf16 x load -> plain matmuls,
    no PE transposes), store f32 tables in DRAM scratch.
  * SWDGE dma_gather costs ~8 ns/index regardless of element size, so
    minimize INDEX COUNT: each core's edges are split into 4 groups by
    (row>=NT/2, col>=NT/2); each group needs exactly ONE head gather window
    and ONE tail gather window (int16-addressable), eliminating the lo/hi
    double-gather of v1.  Groups are padded to a fixed cap with ea=0 edges,
    which contribute exact zeros to the BN sums.
  * BN partials AllReduce'd across cores between pass 1 and pass 2 as in v1.
"""

import numpy as np
import ml_dtypes

import concourse.bass as bass
import concourse.mybir as mybir
import concourse.tile as tile
from concourse import bacc
from concourse.masks import make_identity

P = 128
C = 128
BN_EPS = 1e-5

N_CORES = 8
N_NODES = 40000
N_EDGES = 640000
E_SHARD = N_EDGES // N_CORES      # 80000

NTAB = 40960                      # table rows (node count padded to x128)
NODE_SPLIT = 20000                # group threshold on node id
WIN = 32768                       # int16 gather window
HI_BASE = 8192                    # hi-window base (span-aligned)

GCAP = 20864                      # padded edges per group (163 k-blocks)
E_PAD = 4 * GCAP                  # padded edges per core = 83456
CHUNK = 3200                      # edges per gather chunk (25 k-blocks)
SUB_KB = 4                        # k-blocks per compute sub-step
PIECE = 8192                      # xT transpose-load piece (nodes)
B8 = 16                           # H/T store batch (k-blocks per DMA)
# tables are stored p-major within spans of B8*128 rows: node n lives at
# table row pi(n) = (n//span)*span + (n%128)*B8 + (n%span)//128, so each
# partition's table-store DMA is one contiguous run.  Windows are
# span-aligned, and the host computes gather indices in pi space.

F32 = mybir.dt.float32
BF16 = mybir.dt.bfloat16
I16 = mybir.dt.int16
AF = mybir.ActivationFunctionType
ALU = mybir.AluOpType


def _group_chunks(gcap):
    """chunk sizes for one group: CHUNK-sized plus one remainder."""
    assert gcap % P == 0
    out = []
    e0 = 0
    while e0 < gcap:
        ch = min(CHUNK, gcap - e0)
        out.append(ch)
        e0 += ch
    return out


def _plan(gcap):
    chunks = _group_chunks(gcap)
    nsub_g = sum((ch // P + SUB_KB - 1) // SUB_KB for ch in chunks)
    return chunks, nsub_g


def build_nc(n_cores=N_CORES, gcap=GCAP, ntab=NTAB, nsplit=NODE_SPLIT,
             win=WIN, hibase=HI_BASE, n_edges_total=N_EDGES, no_cc=False):
    chunks, nsub_g = _plan(gcap)
    nchunk_g = len(chunks)
    nchunk = 4 * nchunk_g
    nsub = 4 * nsub_g
    smax = max(chunks) // 16
    e_pad = 4 * gcap
    npiece = ntab // PIECE

    nc = bacc.Bacc(None, num_devices=n_cores)
    xb_t = nc.dram_tensor("xb", [ntab, C], BF16, kind="ExternalInput")
    ea_t = nc.dram_tensor("ea", [e_pad, C], F32, kind="ExternalInput")
    # idxpack[chunk, j, :, :]: j = 0 -> head, 1 -> tail
    idx_t = nc.dram_tensor("idxpack", [nchunk, 2, P, smax], I16,
                           kind="ExternalInput")
    ws_t = nc.dram_tensor("w_self", [C, C], F32, kind="ExternalInput")
    whtb_t = nc.dram_tensor("whtb", [C, 2 * C], BF16, kind="ExternalInput")
    gm_t = nc.dram_tensor("gamma", [C, 1], F32, kind="ExternalInput")
    bt_t = nc.dram_tensor("beta", [C, 1], F32, kind="ExternalInput")
    out_t = nc.dram_tensor("out", [C, e_pad], F32, kind="ExternalOutput")

    with tile.TileContext(nc, num_cores=n_cores) as tc:
        with (
            tc.tile_pool(name="constp", bufs=1) as constp,
            tc.tile_pool(name="dramp", bufs=1, space="DRAM") as dramp,
        ):
            identity = constp.tile([P, P], F32)
            make_identity(nc, identity[:])
            identity_bf = constp.tile([P, P], BF16)
            nc.vector.tensor_copy(identity_bf[:], identity[:])
            ws_sb = constp.tile([P, C], F32)
            nc.sync.dma_start(ws_sb[:], ws_t[:, :])
            whtb = constp.tile([P, 2 * C], BF16)
            nc.sync.dma_start(whtb[:], whtb_t[:, :])
            gamma_sb = constp.tile([P, 1], F32)
            nc.sync.dma_start(gamma_sb[:], gm_t[:, :])
            beta_sb = constp.tile([P, 1], F32)
            nc.sync.dma_start(beta_sb[:], bt_t[:, :])

            sum_cols = constp.tile([P, nsub], F32)
            sq_cols = constp.tile([P, nsub], F32)

            ht_tab = dramp.tile([ntab, 2 * C], F32)
            op_scratch = dramp.tile([nsub, P, SUB_KB, P], BF16)

            # ---- node-table precompute: H = x @ whb, T = x @ wtb ----
            with (
                tc.tile_pool(name="prep", bufs=2) as prep,
                tc.tile_pool(name="prps", bufs=2, space="PSUM") as prps,
            ):
                for pc in range(npiece):
                    xT = prep.tile([P, PIECE], BF16, tag="xT")
                    nc.sync.dma_start_transpose(
                        xT[:], xb_t[pc * PIECE:(pc + 1) * PIECE, :]
                    )
                    nblk = PIECE // P
                    for b8 in range(0, nblk, B8):
                        ht_sb = prep.tile([P, B8, 2 * C], F32, tag="htsb")
                        for b in range(b8, b8 + B8):
                            ht_ps = prps.tile([P, 2 * C], F32, tag="htps",
                                              bufs=2)
                            nc.tensor.matmul(
                                ht_ps[:], lhsT=xT[:, b * P:(b + 1) * P],
                                rhs=whtb[:], start=True, stop=True,
                            )
                            if b % 2 == 0:
                                nc.scalar.copy(ht_sb[:, b - b8, :], ht_ps[:])
                            else:
                                nc.vector.tensor_copy(ht_sb[:, b - b8, :],
                                                      ht_ps[:])
                        r0 = pc * PIECE + b8 * P
                        nc.sync.dma_start(
                            ht_tab[r0:r0 + B8 * P, :].rearrange(
                                "(p b) c -> p b c", p=P),
                            ht_sb[:],
                        )

            # ---- pass 1 over 4 groups ----
            t_idx = 0
            with (
                tc.tile_pool(name="chunkp", bufs=3) as chunkp,
                tc.tile_pool(name="subp", bufs=3) as subp,
                tc.tile_pool(name="psp", bufs=2, space="PSUM") as psp,
            ):
                for g in range(4):
                    hw0 = hibase if (g >> 1) else 0
                    tw0 = hibase if (g & 1) else 0
                    h_win = ht_tab[hw0:hw0 + win, 0:C]
                    t_win = ht_tab[tw0:tw0 + win, C:2 * C]
                    e0 = g * gcap
                    for ci, ch in enumerate(chunks):
                        gci = g * nchunk_g + ci
                        K = ch // P
                        S = ch // 16
                        idx = chunkp.tile([P, 2, smax], I16, tag="idx")
                        nc.sync.dma_start(
                            idx[:, :, 0:S],
                            idx_t[gci, :, :, 0:S].rearrange("j p s -> p j s"),
                        )
                        gh = chunkp.tile([P, K, C], F32, tag="gh")
                        nc.gpsimd.dma_gather(
                            out_ap=gh[:], in_ap=h_win,
                            idxs_ap=idx[:, 0, 0:S],
                            num_idxs=ch, num_idxs_reg=ch, elem_size=C,
                            elem_step=2 * C,
                            single_packet=False,
                        )
                        gt = chunkp.tile([P, K, C], F32, tag="gt")
                        nc.gpsimd.dma_gather(
                            out_ap=gt[:], in_ap=t_win,
                            idxs_ap=idx[:, 1, 0:S],
                            num_idxs=ch, num_idxs_reg=ch, elem_size=C,
                            elem_step=2 * C,
                            single_packet=False,
                        )
                        ea_c = chunkp.tile([P, K, C], F32, tag="eac")
                        nc.sync.dma_start(
                            ea_c[:],
                            ea_t[e0:e0 + ch, :].rearrange(
                                "(k p) c -> p k c", p=P),
                        )
                        e0 += ch

                        for k0 in range(0, K, SUB_KB):
                            kb = min(SUB_KB, K - k0)
                            eaT_ps = psp.tile([P, kb, P], F32, tag="tre",
                                              bufs=2)
                            s_ps = psp.tile([P, kb, P], F32, tag="trs", bufs=2)
                            for j in range(kb):
                                nc.tensor.transpose(
                                    eaT_ps[:, j, :], ea_c[:, k0 + j, :],
                                    identity[:],
                                )
                                nc.tensor.matmul(
                                    s_ps[:, j, :], lhsT=gh[:, k0 + j, :],
                                    rhs=identity[:], is_transpose=True,
                                    start=True, stop=False,
                                )
                                nc.tensor.matmul(
                                    s_ps[:, j, :], lhsT=gt[:, k0 + j, :],
                                    rhs=identity[:], is_transpose=True,
                                    start=False, stop=True,
                                )
                            eaT = subp.tile([P, kb, P], F32, tag="eaT")
                            nc.vector.tensor_copy(eaT[:], eaT_ps[:])
                            o_ps = psp.tile([P, kb, P], F32, tag="ops", bufs=2)
                            nc.tensor.matmul(
                                o_ps[:], lhsT=ws_sb[:], rhs=eaT[:],
                                start=True, stop=True,
                            )
                            # a = 1 + (H[row] + T[col])   (s accumulated in PSUM)
                            a1 = subp.tile([P, kb, P], F32, tag="a1")
                            nc.scalar.activation(a1[:], s_ps[:], AF.Copy,
                                                 bias=1.0)
                            m = subp.tile([P, kb, P], F32, tag="m")
                            nc.vector.tensor_tensor(m[:], o_ps[:], a1[:],
                                                    op=ALU.mult)
                            opT = subp.tile([P, kb, P], BF16, tag="opT")
                            nc.vector.tensor_tensor(opT[:], m[:], eaT[:],
                                                    op=ALU.add)
                            sqt = subp.tile([P, kb, P], F32, tag="sqt")
                            nc.scalar.activation(
                                sqt[:], opT[:], AF.Square,
                                accum_out=sq_cols[:, t_idx:t_idx + 1],
                            )
                            nc.vector.tensor_reduce(
                                sum_cols[:, t_idx:t_idx + 1], opT[:],
                                axis=mybir.AxisListType.XY, op=ALU.add,
                            )
                            nc.scalar.dma_start(
                                op_scratch[t_idx, :, 0:kb, :], opT[:]
                            )
                            t_idx += 1
            assert t_idx == nsub

            # ---- BN stats all-reduce + scale/shift ----
            stats2 = constp.tile([P, 2], F32)
            nc.vector.tensor_reduce(
                stats2[:, 0:1], sum_cols[:], axis=mybir.AxisListType.X,
                op=ALU.add,
            )
            nc.vector.tensor_reduce(
                stats2[:, 1:2], sq_cols[:], axis=mybir.AxisListType.X,
                op=ALU.add,
            )
            if no_cc:
                statsg = stats2
            else:
                cc_in = dramp.tile([P, 2], F32)
                nc.sync.dma_start(cc_in[:], stats2[:])
                cc_addr = "Shared" if n_cores > 4 else "Local"
                cc_out = dramp.tile([P, 2], F32, addr_space=cc_addr)
                nc.gpsimd.collective_compute(
                    "AllReduce",
                    ALU.add,
                    replica_groups=[list(range(n_cores))],
                    ins=[cc_in[:].opt()],
                    outs=[cc_out[:].opt()],
                )
                statsg = constp.tile([P, 2], F32)
                nc.sync.dma_start(statsg[:], cc_out[:])

            inv_e = 1.0 / float(n_edges_total)
            mean = constp.tile([P, 1], F32)
            nc.scalar.mul(mean[:], statsg[:, 0:1], inv_e)
            ex2 = constp.tile([P, 1], F32)
            nc.scalar.mul(ex2[:], statsg[:, 1:2], inv_e)
            msq = constp.tile([P, 1], F32)
            nc.vector.tensor_tensor(msq[:], mean[:], mean[:], op=ALU.mult)
            var = constp.tile([P, 1], F32)
            nc.vector.tensor_tensor(var[:], ex2[:], msq[:], op=ALU.subtract)
            eps_sb = constp.tile([P, 1], F32)
            nc.vector.memset(eps_sb[:], BN_EPS)
            std = constp.tile([P, 1], F32)
            nc.scalar.activation(std[:], var[:], AF.Sqrt, bias=eps_sb[:])
            rstd = constp.tile([P, 1], F32)
            nc.vector.reciprocal(rstd[:], std[:])
            scale = constp.tile([P, 1], F32)
            nc.vector.tensor_tensor(scale[:], gamma_sb[:], rstd[:],
                                    op=ALU.mult)
            mscale = constp.tile([P, 1], F32)
            nc.vector.tensor_tensor(mscale[:], mean[:], scale[:], op=ALU.mult)
            shift = constp.tile([P, 1], F32)
            nc.vector.tensor_tensor(shift[:], beta_sb[:], mscale[:],
                                    op=ALU.subtract)

            # ---- pass 2: normalize + relu + transpose back ----
            t_idx = 0
            with tc.tile_pool(name="p2p", bufs=6) as p2p:
                e0 = 0
                for g in range(4):
                    for ch in chunks:
                        K = ch // P
                        for k0 in range(0, K, SUB_KB):
                            kb = min(SUB_KB, K - k0)
                            opn = p2p.tile([P, kb, P], BF16, tag="opn")
                            nc.sync.dma_start(
                                opn[:], op_scratch[t_idx, :, 0:kb, :]
                            )
                            nrm = p2p.tile([P, kb, P], F32, tag="nrm")
                            nc.scalar.activation(
                                nrm[:], opn[:], AF.Relu, bias=shift[:],
                                scale=scale[:],
                            )
                            a = e0 + k0 * P
                            nc.scalar.dma_start(
                                out_t[:, a:a + kb * P], nrm[:]
                            )
                            t_idx += 1
                        e0 += ch
            assert t_idx == nsub

    if not nc.is_finalized():
        nc.finalize()
    return nc


def _wrap16(a):
    """[n] int array -> dma_gather idx layout [128, n//16] int16."""
    s = a.shape[0] // 16
    w = a.reshape(s, 16).T.astype(np.int16)
    return np.tile(w, (8, 1))


def make_in_maps(x, edge_index, edge_attr, w_self, w_h, w_t, gamma, beta_bn):
    x = np.asarray(x, dtype=np.float32)
    xb = np.zeros((NTAB, C), dtype=ml_dtypes.bfloat16)
    xb[:N_NODES] = x.astype(ml_dtypes.bfloat16)

    ea = np.asarray(edge_attr, dtype=np.float32)
    ei = np.asarray(edge_index)
    row = ei[0].astype(np.int64)
    col = ei[1].astype(np.int64)
    ws = np.ascontiguousarray(np.asarray(w_self, dtype=np.float32))
    whtb = np.ascontiguousarray(np.concatenate([
        (0.5 * np.asarray(w_h, dtype=np.float32)).astype(ml_dtypes.bfloat16),
        (0.5 * np.asarray(w_t, dtype=np.float32)).astype(ml_dtypes.bfloat16),
    ], axis=1))
    gm = np.ascontiguousarray(np.asarray(gamma, np.float32).reshape(C, 1))
    bt = np.ascontiguousarray(np.asarray(beta_bn, np.float32).reshape(C, 1))

    chunks, _ = _plan(GCAP)
    nchunk_g = len(chunks)
    smax = max(chunks) // 16

    in_maps = []
    placements = []   # per core: global edge ids at each padded slot (-1 pad)
    for k in range(N_CORES):
        sl = slice(k * E_SHARD, (k + 1) * E_SHARD)
        r = row[sl]
        c = col[sl]
        gid = (r >= NODE_SPLIT).astype(np.int64) * 2 + \
              (c >= NODE_SPLIT).astype(np.int64)

        span = B8 * P

        def pi(n):
            return (n // span) * span + (n % P) * B8 + (n % span) // P

        ea_pad = np.zeros((E_PAD, C), dtype=np.float32)
        hidx = np.zeros(E_PAD, dtype=np.int64)
        tidx = np.zeros(E_PAD, dtype=np.int64)
        place = np.full(E_PAD, -1, dtype=np.int64)
        for g in range(4):
            idsg = np.nonzero(gid == g)[0]
            n = idsg.shape[0]
            assert n <= GCAP, f"group {g} count {n} exceeds cap {GCAP}"
            b = g * GCAP
            place[b:b + n] = idsg + k * E_SHARD
            ea_pad[b:b + n] = ea[sl][idsg]
            hw0 = HI_BASE if (g >> 1) else 0
            tw0 = HI_BASE if (g & 1) else 0
            hidx[b:b + n] = pi(r[idsg]) - hw0
            tidx[b:b + n] = pi(c[idsg]) - tw0

        packs = np.zeros((4 * nchunk_g, 2, P, smax), dtype=np.int16)
        for g in range(4):
            e0 = g * GCAP
            for ci, ch in enumerate(chunks):
                s = ch // 16
                gci = g * nchunk_g + ci
                packs[gci, 0, :, 0:s] = _wrap16(hidx[e0:e0 + ch])
                packs[gci, 1, :, 0:s] = _wrap16(tidx[e0:e0 + ch])
                e0 += ch

        in_maps.append({
            "xb": xb,
            "ea": ea_pad,
            "idxpack": packs,
            "w_self": ws,
            "whtb": whtb,
            "gamma": gm,
            "beta": bt,
        })
        placements.append(place)
    return in_maps, placements


_NC_CACHE = {}


def _get_nc():
    if "nc" not in _NC_CACHE:
        _NC_CACHE["nc"] = build_nc()
    return _NC_CACHE["nc"]


def run(inputs, trace=False, **kwargs):
    from concourse.bass_utils import run_bass_kernel_spmd

    nc = _get_nc()
    in_maps, placements = make_in_maps(
        inputs["x"], inputs["edge_index"], inputs["edge_attr"],
        inputs["w_self"], inputs["w_h"], inputs["w_t"],
        inputs["gamma"], inputs["beta_bn"],
    )
    res = run_bass_kernel_spmd(
        nc, in_maps, core_ids=list(range(N_CORES)), trace=trace, **kwargs
    )
    out = np.empty((N_EDGES, C), dtype=np.float32)
    for k in range(N_CORES):
        dev = np.ascontiguousarray(res.results[k]["out"].T)
        place = placements[k]
        valid = place >= 0
        out[place[valid]] = dev[valid]
    return out, res


def kernel(**inputs):
    out, _ = run(inputs, trace=False)
    return out


# revision 3
# speedup vs baseline: 1.0188x; 1.0188x over previous
"""EdgeConv GNN message passing on 8 TRN2 cores — v2.

Math per edge e (endpoints row[e], col[e]):
    out0 = edge_attr @ w_self
    out  = out0 * (1 + 0.5*x[row]@w_h + 0.5*x[col]@w_t) + edge_attr
    out  = relu(batchnorm(out))        # BN stats over ALL edges

Key ideas vs v1:
  * x[row] @ w_h == (x @ w_h)[row]: precompute node tables H = 0.5*x@w_h,
    T = 0.5*x@w_t on device (xbar-transposed bf16 x load -> plain matmuls,
    no PE transposes), store f32 tables in DRAM scratch.
  * SWDGE dma_gather costs ~8 ns/index regardless of element size, so
    minimize INDEX COUNT: each core's edges are split into 4 groups by
    (row>=NT/2, col>=NT/2); each group needs exactly ONE head gather window
    and ONE tail gather window (int16-addressable), eliminating the lo/hi
    double-gather of v1.  Groups are padded to a fixed cap with ea=0 edges,
    which contribute exact zeros to the BN sums.
  * BN partials AllReduce'd across cores between pass 1 and pass 2 as in v1.
"""

import numpy as np
import ml_dtypes

import concourse.bass as bass
import concourse.mybir as mybir
import concourse.tile as tile
from concourse import bacc
from concourse.masks import make_identity

P = 128
C = 128
BN_EPS = 1e-5

N_CORES = 8
N_NODES = 40000
N_EDGES = 640000
E_SHARD = N_EDGES // N_CORES      # 80000

NTAB = 40960                      # table rows (node count padded to x128)
NODE_SPLIT = 20000                # group threshold on node id
WIN = 32768                       # int16 gather window
HI_BASE = 8192                    # hi-window base (span-aligned)

GCAP = 20480                      # padded edges per group (160 k-blocks)
E_PAD = 4 * GCAP                  # padded edges per core = 83456
CHUNK = 3200                      # edges per gather chunk (25 k-blocks)
SUB_KB = 4                        # k-blocks per compute sub-step
PIECE = 8192                      # xT transpose-load piece (nodes)
B8 = 16                           # H/T store batch (k-blocks per DMA)
# tables are stored p-major within spans of B8*128 rows: node n lives at
# table row pi(n) = (n//span)*span + (n%128)*B8 + (n%span)//128, so each
# partition's table-store DMA is one contiguous run.  Windows are
# span-aligned, and the host computes gather indices in pi space.

F32 = mybir.dt.float32
BF16 = mybir.dt.bfloat16
I16 = mybir.dt.int16
AF = mybir.ActivationFunctionType
ALU = mybir.AluOpType


def _group_chunks(gcap):
    """chunk sizes for one group: CHUNK-sized plus one remainder."""
    assert gcap % P == 0
    out = []
    e0 = 0
    while e0 < gcap:
        ch = min(CHUNK, gcap - e0)
        out.append(ch)
        e0 += ch
    return out


def _plan(gcap):
    chunks = _group_chunks(gcap)
    nsub_g = sum((ch // P + SUB_KB - 1) // SUB_KB for ch in chunks)
    return chunks, nsub_g


def build_nc(n_cores=N_CORES, gcap=GCAP, ntab=NTAB, nsplit=NODE_SPLIT,
             win=WIN, hibase=HI_BASE, n_edges_total=N_EDGES, no_cc=False):
    chunks, nsub_g = _plan(gcap)
    nchunk_g = len(chunks)
    nchunk = 4 * nchunk_g
    nsub = 4 * nsub_g
    smax = max(chunks) // 16
    e_pad = 4 * gcap
    npiece = ntab // PIECE

    nc = bacc.Bacc(None, num_devices=n_cores)
    xb_t = nc.dram_tensor("xb", [ntab, C], BF16, kind="ExternalInput")
    ea_t = nc.dram_tensor("ea", [e_pad, C], F32, kind="ExternalInput")
    # idxpack[chunk, j, :, :]: j = 0 -> head, 1 -> tail
    idx_t = nc.dram_tensor("idxpack", [nchunk, 2, P, smax], I16,
                           kind="ExternalInput")
    ws_t = nc.dram_tensor("w_self", [C, C], F32, kind="ExternalInput")
    whtb_t = nc.dram_tensor("whtb", [C, 2 * C], BF16, kind="ExternalInput")
    gm_t = nc.dram_tensor("gamma", [C, 1], F32, kind="ExternalInput")
    bt_t = nc.dram_tensor("beta", [C, 1], F32, kind="ExternalInput")
    out_t = nc.dram_tensor("out", [C, e_pad], BF16, kind="ExternalOutput")

    with tile.TileContext(nc, num_cores=n_cores) as tc:
        with (
            tc.tile_pool(name="constp", bufs=1) as constp,
            tc.tile_pool(name="dramp", bufs=1, space="DRAM") as dramp,
        ):
            identity = constp.tile([P, P], F32)
            make_identity(nc, identity[:])
            identity_bf = constp.tile([P, P], BF16)
            nc.vector.tensor_copy(identity_bf[:], identity[:])
            ws_sb = constp.tile([P, C], F32)
            nc.sync.dma_start(ws_sb[:], ws_t[:, :])
            whtb = constp.tile([P, 2 * C], BF16)
            nc.sync.dma_start(whtb[:], whtb_t[:, :])
            gamma_sb = constp.tile([P, 1], F32)
            nc.sync.dma_start(gamma_sb[:], gm_t[:, :])
            beta_sb = constp.tile([P, 1], F32)
            nc.sync.dma_start(beta_sb[:], bt_t[:, :])

            sum_cols = constp.tile([P, nsub], F32)
            sq_cols = constp.tile([P, nsub], F32)

            ht_tab = dramp.tile([ntab, 2 * C], F32)
            op_scratch = dramp.tile([nsub, P, SUB_KB, P], BF16)

            # ---- node-table precompute: H = x @ whb, T = x @ wtb ----
            with (
                tc.tile_pool(name="prep", bufs=2) as prep,
                tc.tile_pool(name="prps", bufs=2, space="PSUM") as prps,
            ):
                for pc in range(npiece):
                    xT = prep.tile([P, PIECE], BF16, tag="xT")
                    nc.sync.dma_start_transpose(
                        xT[:], xb_t[pc * PIECE:(pc + 1) * PIECE, :]
                    )
                    nblk = PIECE // P
                    for b8 in range(0, nblk, B8):
                        ht_sb = prep.tile([P, B8, 2 * C], F32, tag="htsb")
                        for b in range(b8, b8 + B8):
                            ht_ps = prps.tile([P, 2 * C], F32, tag="htps",
                                              bufs=2)
                            nc.tensor.matmul(
                                ht_ps[:], lhsT=xT[:, b * P:(b + 1) * P],
                                rhs=whtb[:], start=True, stop=True,
                            )
                            if b % 2 == 0:
                                nc.scalar.copy(ht_sb[:, b - b8, :], ht_ps[:])
                            else:
                                nc.vector.tensor_copy(ht_sb[:, b - b8, :],
                                                      ht_ps[:])
                        r0 = pc * PIECE + b8 * P
                        nc.sync.dma_start(
                            ht_tab[r0:r0 + B8 * P, :].rearrange(
                                "(p b) c -> p b c", p=P),
                            ht_sb[:],
                        )

            # ---- pass 1 over 4 groups ----
            t_idx = 0
            with (
                tc.tile_pool(name="chunkp", bufs=3) as chunkp,
                tc.tile_pool(name="subp", bufs=3) as subp,
                tc.tile_pool(name="psp", bufs=2, space="PSUM") as psp,
            ):
                for g in range(4):
                    hw0 = hibase if (g >> 1) else 0
                    tw0 = hibase if (g & 1) else 0
                    h_win = ht_tab[hw0:hw0 + win, 0:C]
                    t_win = ht_tab[tw0:tw0 + win, C:2 * C]
                    e0 = g * gcap
                    for ci, ch in enumerate(chunks):
                        gci = g * nchunk_g + ci
                        K = ch // P
                        S = ch // 16
                        idx = chunkp.tile([P, 2, smax], I16, tag="idx")
                        nc.sync.dma_start(
                            idx[:, :, 0:S],
                            idx_t[gci, :, :, 0:S].rearrange("j p s -> p j s"),
                        )
                        gh = chunkp.tile([P, K, C], F32, tag="gh")
                        nc.gpsimd.dma_gather(
                            out_ap=gh[:], in_ap=h_win,
                            idxs_ap=idx[:, 0, 0:S],
                            num_idxs=ch, num_idxs_reg=ch, elem_size=C,
                            elem_step=2 * C,
                            single_packet=False,
                        )
                        gt = chunkp.tile([P, K, C], F32, tag="gt")
                        nc.gpsimd.dma_gather(
                            out_ap=gt[:], in_ap=t_win,
                            idxs_ap=idx[:, 1, 0:S],
                            num_idxs=ch, num_idxs_reg=ch, elem_size=C,
                            elem_step=2 * C,
                            single_packet=False,
                        )
                        ea_c = chunkp.tile([P, K, C], F32, tag="eac")
                        nc.sync.dma_start(
                            ea_c[:],
                            ea_t[e0:e0 + ch, :].rearrange(
                                "(k p) c -> p k c", p=P),
                        )
                        e0 += ch

                        for k0 in range(0, K, SUB_KB):
                            kb = min(SUB_KB, K - k0)
                            eaT_ps = psp.tile([P, kb, P], F32, tag="tre",
                                              bufs=2)
                            s_ps = psp.tile([P, kb, P], F32, tag="trs", bufs=2)
                            for j in range(kb):
                                nc.tensor.transpose(
                                    eaT_ps[:, j, :], ea_c[:, k0 + j, :],
                                    identity[:],
                                )
                                nc.tensor.matmul(
                                    s_ps[:, j, :], lhsT=gh[:, k0 + j, :],
                                    rhs=identity[:], is_transpose=True,
                                    start=True, stop=False,
                                )
                                nc.tensor.matmul(
                                    s_ps[:, j, :], lhsT=gt[:, k0 + j, :],
                                    rhs=identity[:], is_transpose=True,
                                    start=False, stop=True,
                                )
                            eaT = subp.tile([P, kb, P], F32, tag="eaT")
                            nc.vector.tensor_copy(eaT[:], eaT_ps[:])
                            o_ps = psp.tile([P, kb, P], F32, tag="ops", bufs=2)
                            nc.tensor.matmul(
                                o_ps[:], lhsT=ws_sb[:], rhs=eaT[:],
                                start=True, stop=True,
                            )
                            # a = 1 + (H[row] + T[col])   (s accumulated in PSUM)
                            a1 = subp.tile([P, kb, P], F32, tag="a1")
                            nc.scalar.activation(a1[:], s_ps[:], AF.Copy,
                                                 bias=1.0)
                            m = subp.tile([P, kb, P], F32, tag="m")
                            nc.vector.tensor_tensor(m[:], o_ps[:], a1[:],
                                                    op=ALU.mult)
                            opT = subp.tile([P, kb, P], BF16, tag="opT")
                            nc.vector.tensor_tensor(opT[:], m[:], eaT[:],
                                                    op=ALU.add)
                            sqt = subp.tile([P, kb, P], F32, tag="sqt")
                            nc.scalar.activation(
                                sqt[:], opT[:], AF.Square,
                                accum_out=sq_cols[:, t_idx:t_idx + 1],
                            )
                            nc.vector.tensor_reduce(
                                sum_cols[:, t_idx:t_idx + 1], opT[:],
                                axis=mybir.AxisListType.XY, op=ALU.add,
                            )
                            nc.scalar.dma_start(
                                op_scratch[t_idx, :, 0:kb, :], opT[:]
                            )
                            t_idx += 1
            assert t_idx == nsub

            # ---- BN stats all-reduce + scale/shift ----
            stats2 = constp.tile([P, 2], F32)
            nc.vector.tensor_reduce(
                stats2[:, 0:1], sum_cols[:], axis=mybir.AxisListType.X,
                op=ALU.add,
            )
            nc.vector.tensor_reduce(
                stats2[:, 1:2], sq_cols[:], axis=mybir.AxisListType.X,
                op=ALU.add,
            )
            if no_cc:
                statsg = stats2
            else:
                cc_in = dramp.tile([P, 2], F32)
                nc.sync.dma_start(cc_in[:], stats2[:])
                cc_addr = "Shared" if n_cores > 4 else "Local"
                cc_out = dramp.tile([P, 2], F32, addr_space=cc_addr)
                nc.gpsimd.collective_compute(
                    "AllReduce",
                    ALU.add,
                    replica_groups=[list(range(n_cores))],
                    ins=[cc_in[:].opt()],
                    outs=[cc_out[:].opt()],
                )
                statsg = constp.tile([P, 2], F32)
                nc.sync.dma_start(statsg[:], cc_out[:])

            inv_e = 1.0 / float(n_edges_total)
            mean = constp.tile([P, 1], F32)
            nc.scalar.mul(mean[:], statsg[:, 0:1], inv_e)
            ex2 = constp.tile([P, 1], F32)
            nc.scalar.mul(ex2[:], statsg[:, 1:2], inv_e)
            msq = constp.tile([P, 1], F32)
            nc.vector.tensor_tensor(msq[:], mean[:], mean[:], op=ALU.mult)
            var = constp.tile([P, 1], F32)
            nc.vector.tensor_tensor(var[:], ex2[:], msq[:], op=ALU.subtract)
            eps_sb = constp.tile([P, 1], F32)
            nc.vector.memset(eps_sb[:], BN_EPS)
            std = constp.tile([P, 1], F32)
            nc.scalar.activation(std[:], var[:], AF.Sqrt, bias=eps_sb[:])
            rstd = constp.tile([P, 1], F32)
            nc.vector.reciprocal(rstd[:], std[:])
            scale = constp.tile([P, 1], F32)
            nc.vector.tensor_tensor(scale[:], gamma_sb[:], rstd[:],
                                    op=ALU.mult)
            mscale = constp.tile([P, 1], F32)
            nc.vector.tensor_tensor(mscale[:], mean[:], scale[:], op=ALU.mult)
            shift = constp.tile([P, 1], F32)
            nc.vector.tensor_tensor(shift[:], beta_sb[:], mscale[:],
                                    op=ALU.subtract)

            # ---- pass 2: normalize + relu + transpose back ----
            t_idx = 0
            with tc.tile_pool(name="p2p", bufs=6) as p2p:
                e0 = 0
                for g in range(4):
                    for ch in chunks:
                        K = ch // P
                        for k0 in range(0, K, SUB_KB):
                            kb = min(SUB_KB, K - k0)
                            opn = p2p.tile([P, kb, P], BF16, tag="opn")
                            nc.sync.dma_start(
                                opn[:], op_scratch[t_idx, :, 0:kb, :]
                            )
                            nrm = p2p.tile([P, kb, P], BF16, tag="nrm")
                            nc.scalar.activation(
                                nrm[:], opn[:], AF.Relu, bias=shift[:],
                                scale=scale[:],
                            )
                            a = e0 + k0 * P
                            nc.scalar.dma_start(
                                out_t[:, a:a + kb * P], nrm[:]
                            )
                            t_idx += 1
                        e0 += ch
            assert t_idx == nsub

    if not nc.is_finalized():
        nc.finalize()
    return nc


def _wrap16(a):
    """[n] int array -> dma_gather idx layout [128, n//16] int16."""
    s = a.shape[0] // 16
    w = a.reshape(s, 16).T.astype(np.int16)
    return np.tile(w, (8, 1))


def make_in_maps(x, edge_index, edge_attr, w_self, w_h, w_t, gamma, beta_bn):
    x = np.asarray(x, dtype=np.float32)
    xb = np.zeros((NTAB, C), dtype=ml_dtypes.bfloat16)
    xb[:N_NODES] = x.astype(ml_dtypes.bfloat16)

    ea = np.asarray(edge_attr, dtype=np.float32)
    ei = np.asarray(edge_index)
    row = ei[0].astype(np.int64)
    col = ei[1].astype(np.int64)
    ws = np.ascontiguousarray(np.asarray(w_self, dtype=np.float32))
    whtb = np.ascontiguousarray(np.concatenate([
        (0.5 * np.asarray(w_h, dtype=np.float32)).astype(ml_dtypes.bfloat16),
        (0.5 * np.asarray(w_t, dtype=np.float32)).astype(ml_dtypes.bfloat16),
    ], axis=1))
    gm = np.ascontiguousarray(np.asarray(gamma, np.float32).reshape(C, 1))
    bt = np.ascontiguousarray(np.asarray(beta_bn, np.float32).reshape(C, 1))

    chunks, _ = _plan(GCAP)
    nchunk_g = len(chunks)
    smax = max(chunks) // 16

    in_maps = []
    placements = []   # per core: global edge ids at each padded slot (-1 pad)
    for k in range(N_CORES):
        sl = slice(k * E_SHARD, (k + 1) * E_SHARD)
        r = row[sl]
        c = col[sl]
        gid = (r >= NODE_SPLIT).astype(np.int64) * 2 + \
              (c >= NODE_SPLIT).astype(np.int64)

        span = B8 * P

        def pi(n):
            return (n // span) * span + (n % P) * B8 + (n % span) // P

        ea_pad = np.zeros((E_PAD, C), dtype=np.float32)
        hidx = np.zeros(E_PAD, dtype=np.int64)
        tidx = np.zeros(E_PAD, dtype=np.int64)
        place = np.full(E_PAD, -1, dtype=np.int64)
        for g in range(4):
            idsg = np.nonzero(gid == g)[0]
            n = idsg.shape[0]
            assert n <= GCAP, f"group {g} count {n} exceeds cap {GCAP}"
            b = g * GCAP
            place[b:b + n] = idsg + k * E_SHARD
            ea_pad[b:b + n] = ea[sl][idsg]
            hw0 = HI_BASE if (g >> 1) else 0
            tw0 = HI_BASE if (g & 1) else 0
            hidx[b:b + n] = pi(r[idsg]) - hw0
            tidx[b:b + n] = pi(c[idsg]) - tw0

        packs = np.zeros((4 * nchunk_g, 2, P, smax), dtype=np.int16)
        for g in range(4):
            e0 = g * GCAP
            for ci, ch in enumerate(chunks):
                s = ch // 16
                gci = g * nchunk_g + ci
                packs[gci, 0, :, 0:s] = _wrap16(hidx[e0:e0 + ch])
                packs[gci, 1, :, 0:s] = _wrap16(tidx[e0:e0 + ch])
                e0 += ch

        in_maps.append({
            "xb": xb,
            "ea": ea_pad,
            "idxpack": packs,
            "w_self": ws,
            "whtb": whtb,
            "gamma": gm,
            "beta": bt,
        })
        placements.append(place)
    return in_maps, placements


_NC_CACHE = {}


def _get_nc():
    if "nc" not in _NC_CACHE:
        _NC_CACHE["nc"] = build_nc()
    return _NC_CACHE["nc"]


def run(inputs, trace=False, **kwargs):
    from concourse.bass_utils import run_bass_kernel_spmd

    nc = _get_nc()
    in_maps, placements = make_in_maps(
        inputs["x"], inputs["edge_index"], inputs["edge_attr"],
        inputs["w_self"], inputs["w_h"], inputs["w_t"],
        inputs["gamma"], inputs["beta_bn"],
    )
    res = run_bass_kernel_spmd(
        nc, in_maps, core_ids=list(range(N_CORES)), trace=trace, **kwargs
    )
    out = np.empty((N_EDGES, C), dtype=np.float32)
    for k in range(N_CORES):
        dev = np.ascontiguousarray(res.results[k]["out"].T,
                                   dtype=np.float32)
        place = placements[k]
        valid = place >= 0
        out[place[valid]] = dev[valid]
    return out, res


def kernel(**inputs):
    out, _ = run(inputs, trace=False)
    return out
